# revision 1
# baseline (speedup 1.0000x reference)
# Bass/Tile Trainium2 kernel for batched multi-head causal self-attention.
#
# Problem: x[B=2,T=2048,C=1024], 16 heads (hd=64), causal softmax attention,
# output projection. Full (unsharded) inputs in, full output out.
#
# Sharding (Megatron-style): 8 cores = 2 batch groups x 4 head groups.
# Core i handles batch b = i // 4 and heads [4*(i%4) : 4*(i%4)+4).
# Each core computes Q/K/V projections for its 4 heads, causal attention,
# and a partial output projection (contribution of its heads).  The host
# sums the 4 partials per batch (the Megatron all-reduce) and adds bias.
#
# On-device layout notes:
#   - Everything is kept "transposed" (feature dim on partitions):
#     xT [C, T], QT/KT [64, T] per head.  Heads come in pairs packed on
#     the 128 partitions (even head at [0:64], odd head at [64:128]); the
#     K=64 S^T matmuls of a pair use explicit tile_position row groups so
#     they can run concurrently on disjoint PE quadrants.
#   - V is stored [T, 64] per head augmented with a ones column (V') so
#     the P@V matmul also produces the softmax denominator (row 64).
#   - Softmax runs without max-subtraction (scores are bounded ~|10|, exp
#     is safe in fp32), so no partition-dim reductions are ever needed.
#   - Causal masking: k-tiles strictly above the diagonal are skipped;
#     tiles crossing the diagonal get a 128x128 triangular mask multiply
#     and a column-restricted P@V matmul.
#   - QKV matmuls and all attention internals (x, Wq/Wk/Wv, QT/KT/V'/P^T)
#     are bf16; the normalized O^T and Wp stay fp32 and the projection
#     runs in float32r (full fp32 data at ~full PE rate).
#   - Softmax denominators: per-chain rows are DMA'd into a [8, 512]
#     collection tile (DMA may write any partition; engines may not), one
#     batched DVE reciprocal serves a whole head-pair, and GpSimd
#     partition-broadcast + DVE multiply apply the normalization.

import numpy as np

import concourse.bass as bass
import concourse.tile as tile
from concourse import bacc, mybir
from concourse import bass_utils

F32 = mybir.dt.float32
F32R = mybir.dt.float32r
BF16 = mybir.dt.bfloat16
ATT_DT = BF16   # dtype of attention operands (qt/kt/v'/pt/mask)

B, T, C, H = 2, 2048, 1024, 16
HD = C // H            # 64 head dim
NCORES = 8
HPC = 4                # heads per core
DSEL = HPC * HD        # 256 feature dims per core
NTT = T // 128         # 16 t-tiles of 128
NTB = T // 512         # 4 t-blocks of 512
NCC = C // 128         # 8 c-chunks of 128
NQB = T // 512         # 4 q-blocks of 512


def build_program(do_attn=True, do_proj=True, attn_sel=None, dump_ot=False):
    nc = bacc.Bacc("TRN2", target_bir_lowering=False, debug=False)

    # host-prepared "SBUF images": [128 partitions, ...] with long
    # contiguous per-partition lines for efficient DMA
    xT = nc.dram_tensor("xT", [128, NCC, T], BF16, kind="ExternalInput").ap()
    wqT = nc.dram_tensor("wqT", [128, NCC * DSEL], BF16, kind="ExternalInput").ap()
    wkT = nc.dram_tensor("wkT", [128, NCC * DSEL], BF16, kind="ExternalInput").ap()
    wvT = nc.dram_tensor("wvT", [128, NCC * DSEL], BF16, kind="ExternalInput").ap()
    wpT = nc.dram_tensor("wpT", [128, 2 * C], F32R, kind="ExternalInput").ap()
    maskd = nc.dram_tensor("maskd", [128, 128], ATT_DT, kind="ExternalInput").ap()
    out_p = nc.dram_tensor("out_p", [T, C], F32, kind="ExternalOutput").ap()


    with tile.TileContext(nc) as tc:
        with (
            tc.tile_pool(name="consts", bufs=1) as consts,
            tc.tile_pool(name="persist", bufs=1) as persist,
            tc.tile_pool(name="xin", bufs=10) as xin,
            tc.tile_pool(name="pt", bufs=8) as ptpool,
            tc.tile_pool(name="norm", bufs=12) as norm,
            tc.tile_pool(name="outst", bufs=4) as outst,
            tc.tile_pool(name="pa", bufs=4, space="PSUM") as pa,
        ):
            # ---- constants / weights -------------------------------------
            wq_sb = consts.tile([128, NCC, DSEL], BF16, tag="wq")
            wk_sb = consts.tile([128, NCC, DSEL], BF16, tag="wk")
            wv_sb = consts.tile([128, NCC, DSEL], BF16, tag="wv")
            wp_sb = consts.tile([128, 2, C], F32R, tag="wp")
            mk_sb = consts.tile([128, 128], ATT_DT, tag="mk")
            xt_first = xin.tile([128, 1024], BF16, tag="xt", name="xt_first")
            for pg in range(4):
                nc.sync.dma_start(out=xt_first[32 * pg : 32 * pg + 32, :],
                                  in_=xT[32 * pg : 32 * pg + 32, 0, 0:1024])
            for pg in range(4):
                pgs = slice(32 * pg, 32 * pg + 32)
                nc.sync.dma_start(
                    out=wq_sb[pgs].rearrange("p cc d -> p (cc d)"), in_=wqT[pgs])
            for pg in range(4):
                pgs = slice(32 * pg, 32 * pg + 32)
                nc.sync.dma_start(
                    out=wk_sb[pgs].rearrange("p cc d -> p (cc d)"), in_=wkT[pgs])
                nc.sync.dma_start(
                    out=wv_sb[pgs].rearrange("p cc d -> p (cc d)"), in_=wvT[pgs])

            # ---- persistent activations ----------------------------------
            # QT/KT/OT: head pairs packed on partitions ([0:64] even slot,
            # [64:128] odd slot), free dim = t
            qt_sb = persist.tile([128, 2, T], ATT_DT, tag="qt")
            kt_sb = persist.tile([128, 2, T], ATT_DT, tag="kt")
            ot_sb = persist.tile([128, 2, T], F32R, tag="ot")
            # V' per k-tile: 4 heads x (64 V cols + 1 ones col)
            v_sb = persist.tile([128, NTT, HPC * (HD + 1)], ATT_DT, tag="v")

            ones_sb = consts.tile([128, NTT], F32, tag="ones")
            nc.vector.memset(ones_sb[:], 1.0)
            for h in range(HPC):
                nc.vector.tensor_copy(
                    out=v_sb[:, :, h * 65 + 64 : h * 65 + 65],
                    in_=ones_sb[:].rearrange("p (t o) -> p t o", o=1),
                )

            # ---- phase 1: QKV projections --------------------------------
            # QT[d, t] = sum_c wqT[c, d] * xT[c, t]   (and same for K)
            # V[t, d]  = sum_c xT[c, t] * wvT[c, d]
            for tbp in range(NTB // 2):
                xts = []
                for cc in range(NCC):
                    if tbp == 0 and cc == 0:
                        xts.append(xt_first)
                        continue
                    xt = xin.tile([128, 1024], BF16, tag="xt", name=f"xt{cc}")
                    tsp = slice(tbp * 1024, tbp * 1024 + 1024)
                    nc.sync.dma_start(out=xt[0:64, :], in_=xT[0:64, cc, tsp])
                    nc.sync.dma_start(out=xt[64:128, :], in_=xT[64:128, cc, tsp])
                    xts.append(xt)
                for ti in range(2):
                    tb = 2 * tbp + ti
                    ts = slice(tb * 512, tb * 512 + 512)
                    tsl2 = slice(ti * 512, ti * 512 + 512)
                    pq = pa.tile([128, 1024], F32, tag="pa", name="pq")
                    pk = pa.tile([128, 1024], F32, tag="pa", name="pk")
                    pv = pa.tile([128, 1024], F32, tag="pa", name="pv")
                    for cc in range(NCC):
                        xt = xts[cc]
                        st = dict(start=(cc == 0), stop=(cc == NCC - 1))
                        nc.tensor.matmul(pq[:, 0:512], wq_sb[:, cc, 0:128], xt[:, tsl2], **st)
                        nc.tensor.matmul(pq[:, 512:1024], wq_sb[:, cc, 128:256], xt[:, tsl2], **st)
                        nc.tensor.matmul(pk[:, 0:512], wk_sb[:, cc, 0:128], xt[:, tsl2], **st)
                        nc.tensor.matmul(pk[:, 512:1024], wk_sb[:, cc, 128:256], xt[:, tsl2], **st)
                        for tt in range(4):
                            # two 256-col regions share a PSUM bank: only the
                            # first toucher of a bank may set start, only the
                            # last may set stop
                            nc.tensor.matmul(
                                pv[:, tt * 256 : tt * 256 + 256],
                                xt[:, ti * 512 + tt * 128 : ti * 512 + tt * 128 + 128],
                                wv_sb[:, cc, :],
                                start=(cc == 0 and tt % 2 == 0),
                                stop=(cc == NCC - 1 and tt % 2 == 1),
                            )
                    # PSUM -> SBUF (casts to bf16)
                    nc.vector.tensor_copy(
                        out=qt_sb[:, :, ts], in_=pq[:].rearrange("p (s t) -> p s t", s=2)
                    )
                    nc.vector.tensor_copy(
                        out=kt_sb[:, :, ts], in_=pk[:].rearrange("p (s t) -> p s t", s=2)
                    )
                    pv3 = pv[:].rearrange("p (tt d) -> p tt d", tt=4)
                    for h in range(HPC):
                        nc.vector.tensor_copy(
                            out=v_sb[:, tb * 4 : tb * 4 + 4, h * 65 : h * 65 + 64],
                            in_=pv3[:, :, h * 64 : h * 64 + 64],
                        )

            # wp / mask are not needed until later phases: issue their DMAs
            # after phase 1 so they don't delay the first matmuls
            nc.sync.dma_start(out=mk_sb[:], in_=maskd)
            for pg in range(4):
                pgs = slice(32 * pg, 32 * pg + 32)
                nc.sync.dma_start(
                    out=wp_sb[pgs].rearrange("p h c -> p (h c)"), in_=wpT[pgs])

            # ---- phase 2: attention per (head-pair, q-block) -------------
            # S^T[k, q] tiles via K=64 matmuls (pair slots concurrent on PE),
            # exp on ACT, diag-block masking on DVE, P@V' accumulation on PE.
            scale = 1.0 / float(np.sqrt(HD))
            attn = [(hp, qb) for qb in reversed(range(NQB)) for hp in range(2)]
            if not do_attn:
                attn = []
            if attn_sel is not None:
                attn = attn_sel
            den = [persist.tile([4, 512], F32, tag=f"den{i}", name=f"den{i}")
                   for i in range(NQB)]
            rec = [persist.tile([4, 512], F32, tag=f"rec{i}", name=f"rec{i}")
                   for i in range(NQB)]
            psq = {}

            def proj_block(qb):
                for tt in range(4 * qb, 4 * qb + 4):
                    tloc = slice(tt * 128, tt * 128 + 128)
                    pc = pa.tile([128, 1024], F32, tag="pa", name="pc")
                    for cb in range(2):
                        for hpp in range(2):
                            nc.tensor.matmul(
                                pc[:, cb * 512 : cb * 512 + 512],
                                ot_sb[:, hpp, tloc],
                                wp_sb[:, hpp, cb * 512 : cb * 512 + 512],
                                start=(hpp == 0),
                                stop=(hpp == 1),
                            )
                    ob = outst.tile([128, 1024], F32, tag="ob")
                    if tt % 2 == 0:
                        nc.vector.tensor_copy(out=ob[:], in_=pc[:])
                    else:
                        nc.scalar.copy(ob[:], pc[:])
                    for pg in range(4):
                        eng = nc.sync if (tt + pg) % 2 else nc.scalar
                        eng.dma_start(
                            out=out_p[tt * 128 + 32 * pg : tt * 128 + 32 * pg + 32, :],
                            in_=ob[32 * pg : 32 * pg + 32, :])
            for hp, qb in attn:
                qs = slice(qb * 512, qb * 512 + 512)
                n_kt = 4 * (qb + 1)          # k-tiles (128) up to diagonal
                n_g = n_kt // 2              # groups of 2 k-tiles
                po = pa.tile([128, 1024], F32, tag="pa", name="po")
                for g in range(n_g):
                    sg = [pa.tile([128, 1024], F32, tag="pa", name=f"sg{s}")
                          for s in range(2)]
                    pt = [ptpool.tile([128, 1024], ATT_DT, tag="pt", name=f"pt{s}")
                          for s in range(2)]
                    for s in range(2):   # slot-major: exp(s) can start
                        psl = slice(64 * s, 64 * s + 64)
                        for sl in range(2):
                            kt = 2 * g + sl
                            nc.tensor.matmul(
                                sg[s][:, sl * 512 : sl * 512 + 512],
                                kt_sb[psl, hp, kt * 128 : kt * 128 + 128],
                                qt_sb[psl, hp, qs],
                                start=True,
                                stop=True,
                                tile_position=(64 * s, 0),
                            )
                        # exp (no max subtraction; scores bounded)
                        nc.scalar.activation(
                            out=pt[s][:], in_=sg[s][:],
                            func=mybir.ActivationFunctionType.Exp,
                            scale=scale,
                        )
                    for sl in range(2):
                        kt = 2 * g + sl
                        j = kt - 4 * qb      # diag offset, >=0 on diag group
                        roff = 128 * j if j >= 0 else 0
                        for s in range(2):
                            if j >= 0:
                                # triangular mask on the diagonal block
                                dcol = sl * 512 + 128 * j
                                nc.vector.tensor_mul(
                                    pt[s][:, dcol : dcol + 128],
                                    pt[s][:, dcol : dcol + 128],
                                    mk_sb[:],
                                )
                            h = 2 * hp + s
                            nc.tensor.matmul(
                                po[0:65, s * 512 + roff : s * 512 + 512],
                                v_sb[:, kt, h * 65 : h * 65 + 65],
                                pt[s][:, sl * 512 + roff : sl * 512 + 512],
                                start=(kt == 0),
                                stop=(kt == n_kt - 1),
                            )
                # copy O^T + denominator row out of PSUM; normalization is
                # deferred so one batched reciprocal serves the head-pair
                for s in range(2):
                    ps_sb = norm.tile([65, 512], F32, tag="ps",
                                      name=f"ps{hp}{qb}{s}")
                    nc.vector.tensor_copy(out=ps_sb[:], in_=po[0:65, s * 512 : s * 512 + 512])
                    # DMA may read/write any partition row (engines cannot)
                    idx = 2 * hp + s
                    nc.sync.dma_start(out=den[qb][idx : idx + 1, :],
                                      in_=ps_sb[64:65, :])
                    psq[(hp, qb, s)] = ps_sb
                if hp == 1:
                    nc.vector.reciprocal(rec[qb][:], den[qb][:])
                    for hp2 in range(2):
                        for s in range(2):
                            idx = 2 * hp2 + s
                            rc = norm.tile([1, 512], F32, tag="rc", name="rc")
                            rb = norm.tile([64, 512], F32, tag="rb", name="rb")
                            nc.sync.dma_start(out=rc[:], in_=rec[qb][idx : idx + 1, :])
                            nc.gpsimd.partition_broadcast(rb[:], rc[:])
                            nc.vector.tensor_mul(
                                ot_sb[64 * s : 64 * s + 64, hp2,
                                      qb * 512 : qb * 512 + 512],
                                psq[(hp2, qb, s)][0:64, :],
                                rb[:],
                            )

            for qb in reversed(range(NQB)):
                proj_block(qb)

            if dump_ot:
                nc.sync.dma_start(out=out_p[0:128, :],
                                  in_=ot_sb[:, 0, 0:1024].bitcast(F32))
                nc.sync.dma_start(out=out_p[128:256, :],
                                  in_=ot_sb[:, 1, 0:1024].bitcast(F32))

    nc.compile()
    return nc


_NC_CACHE = None


def _get_program():
    global _NC_CACHE
    if _NC_CACHE is None:
        _NC_CACHE = build_program()
    return _NC_CACHE


def make_in_maps(x, Wq, Wk, Wv, Wp):
    import ml_dtypes
    x = np.asarray(x, np.float32)
    Wq = np.asarray(Wq, np.float32)
    Wk = np.asarray(Wk, np.float32)
    Wv = np.asarray(Wv, np.float32)
    Wp = np.asarray(Wp, np.float32)
    maskd = np.triu(np.ones((128, 128), ml_dtypes.bfloat16))  # mask[k,q]=(k<=q)
    in_maps = []
    for core in range(NCORES):
        b, hg = core // 4, core % 4
        sel = slice(hg * DSEL, (hg + 1) * DSEL)
        # SBUF images: [128, cc, ...] with partition index innermost in
        # the original feature dim (feature c -> (cc, p))
        xi = x[b].T.reshape(NCC, 128, T).transpose(1, 0, 2)          # [128, cc, T]
        wqi = Wq[sel, :].T.reshape(NCC, 128, DSEL).transpose(1, 0, 2).reshape(128, NCC * DSEL)
        wki = Wk[sel, :].T.reshape(NCC, 128, DSEL).transpose(1, 0, 2).reshape(128, NCC * DSEL)
        wvi = Wv[sel, :].T.reshape(NCC, 128, DSEL).transpose(1, 0, 2).reshape(128, NCC * DSEL)
        wpi = Wp[:, sel].T.reshape(2, 128, C).transpose(1, 0, 2).reshape(128, 2 * C)
        in_maps.append({
            "xT": np.ascontiguousarray(xi.astype(ml_dtypes.bfloat16)),
            "wqT": np.ascontiguousarray(wqi.astype(ml_dtypes.bfloat16)),
            "wkT": np.ascontiguousarray(wki.astype(ml_dtypes.bfloat16)),
            "wvT": np.ascontiguousarray(wvi.astype(ml_dtypes.bfloat16)),
            "wpT": np.ascontiguousarray(wpi),
            "maskd": maskd,
        })
    return in_maps


def combine_outputs(results, bp):
    parts = [results[i]["out_p"] for i in range(NCORES)]
    out = np.stack([
        parts[0] + parts[1] + parts[2] + parts[3],
        parts[4] + parts[5] + parts[6] + parts[7],
    ])
    return (out + np.asarray(bp, np.float32)).astype(np.float32)


def kernel(x, Wq, Wk, Wv, Wp, bp):
    nc = _get_program()
    in_maps = make_in_maps(x, Wq, Wk, Wv, Wp)
    res = bass_utils.run_bass_kernel_spmd(nc, in_maps, core_ids=list(range(NCORES)))
    return combine_outputs(res.results, bp)

